# revision 3
# baseline (speedup 1.0000x reference)
"""AWQ 4-bit quantized linear (group size 128) on 8 Trainium2 NeuronCores.

Column-parallel: each core owns OUT/8 = 1376 output columns. The host does
layout-only prep (slicing, int4->uint8 nibble widening with the AWQ column
permutation, transposes); all arithmetic — zero-point subtract, scale
multiply, matmul, bias — runs on device.

Per-core device pipeline, for each 128-row block of output columns (o-tile):
  1. DMA the packed-weight rows (uint8 nibbles, o on partitions).
  2. DVE tensor_scalar dual-op dequant: w = nib * s[o] - (z*s)[o]  (fp16 out).
     Scales/zeros vary per (o, group); with o on partitions they are
     per-partition scalars, which tensor_scalar supports natively.
  3. DMA xbar transpose each [o=128, k=128] fp16 tile to [k=128, o=128]
     (matmul needs the contraction dim on partitions).
  4. PE matmul accumulation over the 32 k-groups into PSUM:
     outT[o, m] += w_g[k, o].T @ xT_g[k, m].
  5. ACT/DVE evacuation: out = psum + bias[o] -> fp16, DMA to DRAM.

Head optimizations (the PE stream floor is 704 MMs x 216 ns = 152 us; all
wins beyond that are in not stalling it):
  - ~52 dummy matmuls on zeroed SBUF issued first: lifts the HAM clock gate
    to K=8/8 (~3.4 us of sustained PE activity) before real MMs arrive, and
    keeps the PE warm through the DMA-bound head.
  - consts go on the gpsimd ring so the sync ring starts the otile-0 weight
    chunk immediately; otile-0's first 2 KB/partition of packed weights come
    from a host-contiguous copy (qw0c) for faster DMA.
  - otile 0 dequants in 8 chunks of 4 groups (others: 4 chunks of 8) so the
    first transpose and first matmul land as early as possible.
  - x3 slab loads alternate between the gpsimd and scalar DMA rings.
"""

import os
import sys

import numpy as np

if "/opt/trn_rl_repo" not in sys.path:
    sys.path.insert(0, "/opt/trn_rl_repo")

M, IN, OUT = 1024, 4096, 11008
N_CORES = 8
OC = OUT // N_CORES  # 1376 output columns per core
GS = 128  # quantization group size (== matmul k-tile)
G = IN // GS  # 32 groups
PACK = 8  # int4 values per int32 word
# reference unpacks nibble k to logical column AWQ_REVERSE_ORDER.index(k);
# equivalently logical column j within a word uses shift 4*REV[j]:
REV = np.array([0, 4, 1, 5, 2, 6, 3, 7], dtype=np.uint32)

MM_N = 512  # moving-operand free size per matmul (one PSUM bank of fp32)
N_DUMMY = 52  # PE warmup matmuls (~3.4us cold + ~9.5us warm coverage)

_CACHE = {}


def _unpack_int4(q: np.ndarray) -> np.ndarray:
    """[rows, cols//8] int32 -> [rows, cols] uint8 in 0..15 (AWQ order)."""
    qu = q.view(np.uint32)
    nib = (qu[:, :, None] >> (REV * 4)[None, None, :]) & 0xF
    return nib.reshape(q.shape[0], -1).astype(np.uint8)


def _build(m, k, oc, n_cores):
    import concourse.bacc as bacc
    import concourse.tile as tile
    from concourse import mybir

    F16 = mybir.dt.float16
    F32 = mybir.dt.float32
    U8 = mybir.dt.uint8
    IDENT = mybir.ActivationFunctionType.Identity

    g = k // GS
    n_otiles = (oc + 127) // 128
    n_mch = (m + MM_N - 1) // MM_N

    nc = bacc.Bacc("TRN2", target_bir_lowering=False, debug=False)
    # x pre-swizzled on host to the SBUF layout [partition, group, m] so the
    # load runs with contiguous multi-KB per-partition descriptors
    x3 = nc.dram_tensor("x3", [128, k // GS, m], F16, kind="ExternalInput").ap()
    qw8T = nc.dram_tensor("qw8T", [oc, k], U8, kind="ExternalInput").ap()
    # first 2KB/partition of otile 0 as a contiguous block for the fastest
    # possible first-chunk DMA (dequant critical path at the head)
    qw0c = nc.dram_tensor("qw0c", [128, 2048], U8, kind="ExternalInput").ap()
    n_ot = n_otiles
    # per-otile constants pre-swizzled host-side to partition-major
    # [128, otile, group] so each loads as one contiguous run per partition
    sT = nc.dram_tensor("sT", [128, n_ot, g], F32, kind="ExternalInput").ap()
    z8T = nc.dram_tensor("z8T", [128, n_ot, g], U8, kind="ExternalInput").ap()
    biasT = nc.dram_tensor("biasT", [128, n_ot, 1], F32, kind="ExternalInput").ap()
    outT = nc.dram_tensor("outT", [oc, m], F16, kind="ExternalOutput").ap()
    warm = nc.dram_tensor("warm", [128, 1], F32, kind="ExternalOutput").ap()

    # dequant split 5 DVE / 3 ACT per 8 groups
    # (GPSIMD intentionally unused: it port-muxes with DVE and is ~7x slower)
    deq_pattern = [0, 1, 0, 0, 1, 0, 0, 1]

    with tile.TileContext(nc) as tc:
        with (
            tc.tile_pool(name="x", bufs=1) as xpool,
            tc.tile_pool(name="consts", bufs=1) as cpool,
            tc.tile_pool(name="qw", bufs=4) as qwpool,
            tc.tile_pool(name="wd", bufs=16) as wdpool,
            tc.tile_pool(name="w", bufs=16) as wpool,
            tc.tile_pool(name="ps", bufs=7, space="PSUM") as pspool,
            tc.tile_pool(name="wps", bufs=1, space="PSUM") as wmpool,
            tc.tile_pool(name="o", bufs=4) as opool,
        ):
            # resident transposed activations: [128, g, m]
            xT_sb = xpool.tile([128, g, m], F16)

            # PE warmup: dummy matmuls on a zeroed tile, no data deps. The
            # result is read once at the very end (warm output) so the chain
            # cannot be considered dead.
            dmy_t = cpool.tile([128, MM_N], F16, tag="dmy")
            dps_t = wmpool.tile([128, MM_N], F32, tag="wps")
            nc.gpsimd.memset(dmy_t[:], 0.0)
            for _ in range(N_DUMMY):
                nc.tensor.matmul(
                    dps_t[:], dmy_t[:, :128], dmy_t[:],
                    start=True, stop=True, skip_group_check=True,
                )

            def load_consts():
                s_t = cpool.tile([128, n_otiles, g], F32, tag="sT")
                z8_t = cpool.tile([128, n_otiles, g], U8, tag="z8")
                zf_t = cpool.tile([128, n_otiles, g], F32, tag="zf")
                zs_t = cpool.tile([128, n_otiles, g], F32, tag="zs")
                nzs_t = cpool.tile([128, n_otiles, g], F32, tag="nzs")
                b_t = cpool.tile([128, n_otiles, 1], F32, tag="b")
                nc.gpsimd.dma_start(s_t[:], sT[:])
                nc.gpsimd.dma_start(z8_t[:], z8T[:])
                nc.gpsimd.dma_start(b_t[:], biasT[:])
                nc.vector.tensor_copy(zf_t[:], z8_t[:])
                nc.vector.tensor_tensor(
                    zs_t[:], zf_t[:], s_t[:], mybir.AluOpType.mult
                )
                # on ACT: also warms the activation table before dequant needs it
                nc.scalar.activation(nzs_t[:], zs_t[:], IDENT, scale=-1.0)
                return s_t, zs_t, nzs_t, b_t

            def load_x():
                # alternate rings; slab sizes grow once the head is covered
                for eng, g0, g1 in (
                    (nc.gpsimd, 0, 4),
                    (nc.scalar, 4, 8),
                    (nc.gpsimd, 8, 16),
                    (nc.scalar, 16, 24),
                    (nc.gpsimd, 24, 32),
                ):
                    eng.dma_start(xT_sb[:, g0:g1], x3[:, g0:g1])

            def chunk_groups(ot):
                # otile 0 uses finer chunks so its first matmuls unblock early
                return (4, 8) if ot == 0 else (8, 4)

            def load_qw(ot):
                o0 = ot * 128
                ob = min(128, oc - o0)
                qw_t = qwpool.tile([ob, k], U8, name=f"qw_{ot}", tag="qw")
                if ot == 0:
                    nc.sync.dma_start(qw_t[:, 0:512], qw0c[:, 0:512])
                    nc.sync.dma_start(qw_t[:, 512:2048], qw0c[:, 512:2048])
                    nc.sync.dma_start(qw_t[:, 2048:k], qw8T[o0 : o0 + ob, 2048:k])
                else:
                    nc.sync.dma_start(qw_t[:], qw8T[o0 : o0 + ob])
                return qw_t

            def prep_otile(ot, consts, qw_t):
                o0 = ot * 128
                ob = min(128, oc - o0)
                s_all, zs_all, nzs_all, _ = consts
                s_t = s_all[:ob, ot]
                zs_t = zs_all[:ob, ot]
                nzs_t = nzs_all[:ob, ot]
                qg, nch = chunk_groups(ot)

                # dequant + transpose per chunk (qg groups each)
                w3_qs = []
                for q in range(nch):
                    wd_t = wdpool.tile([ob, qg * GS], F16, tag="wd")
                    for j in range(qg):
                        gi = q * qg + j
                        ksl_q = slice(j * GS, (j + 1) * GS)
                        ksl = slice(gi * GS, (gi + 1) * GS)
                        if deq_pattern[gi % len(deq_pattern)]:
                            nc.scalar.activation(
                                wd_t[:, ksl_q],
                                qw_t[:, ksl],
                                IDENT,
                                bias=nzs_t[:, gi : gi + 1],
                                scale=s_t[:, gi : gi + 1],
                            )
                        else:
                            nc.vector.tensor_scalar(
                                wd_t[:, ksl_q],
                                qw_t[:, ksl],
                                s_t[:, gi : gi + 1],
                                zs_t[:, gi : gi + 1],
                                mybir.AluOpType.mult,
                                mybir.AluOpType.subtract,
                            )
                    # xbar transpose [ob, qg*GS] -> [128, qg, ob]
                    w3_t = wpool.tile([128, qg, ob], F16, tag="w")
                    nc.sync.dma_start_transpose(w3_t[:], wd_t[:])
                    w3_qs.append(w3_t)
                return o0, ob, qg, w3_qs

            def mm_otile(ot, prep, consts, last):
                # interleave the m-chunk accumulation chains per group so each
                # weight chunk is consumed in one dense burst of PE work
                o0, ob, qg, w3_qs = prep
                b_t = consts[3][:ob, ot]
                mslices = [
                    slice(mc * MM_N, min(m, (mc + 1) * MM_N)) for mc in range(n_mch)
                ]
                pss = [
                    pspool.tile([ob, MM_N], F32, name=f"ps_{ot}_{mc}", tag="ps")
                    for mc in range(n_mch)
                ]
                for gi in range(g):
                    for mc in range(n_mch):
                        msl = mslices[mc]
                        nc.tensor.matmul(
                            pss[mc][:, : msl.stop - msl.start],
                            w3_qs[gi // qg][:, gi % qg, :],
                            xT_sb[:, gi, msl],
                            start=(gi == 0),
                            stop=(gi == g - 1),
                        )
                for mc in range(n_mch):
                    msl = mslices[mc]
                    mn = msl.stop - msl.start
                    o_t = opool.tile([ob, MM_N], F16)
                    if not last:
                        # evac split across ACT and DVE to balance engine load
                        if mc == 0:
                            nc.scalar.activation(
                                o_t[:, :mn], pss[mc][:, :mn], IDENT,
                                bias=b_t[:], scale=1.0,
                            )
                        else:
                            nc.vector.tensor_scalar_add(
                                o_t[:, :mn], pss[mc][:, :mn], b_t[:]
                            )
                        nc.gpsimd.dma_start(outT[o0 : o0 + ob, msl], o_t[:, :mn])
                    else:
                        # tail: pipeline evac+store in half-chunks on two rings
                        h = mn // 2
                        for hf in range(2):
                            osl = slice(hf * h, (hf + 1) * h)
                            hsl = slice(msl.start + hf * h, msl.start + (hf + 1) * h)
                            if hf == 0:
                                nc.scalar.activation(
                                    o_t[:, osl], pss[mc][:, osl], IDENT,
                                    bias=b_t[:], scale=1.0,
                                )
                                nc.sync.dma_start(outT[o0 : o0 + ob, hsl], o_t[:, osl])
                            else:
                                nc.vector.tensor_scalar_add(
                                    o_t[:, osl], pss[mc][:, osl], b_t[:]
                                )
                                nc.gpsimd.dma_start(
                                    outT[o0 : o0 + ob, hsl], o_t[:, osl]
                                )

            AHEAD = 3
            # issue order: consts on gpsimd; otile-0/1 packed weights on sync
            # ahead of the (data-dependent) transposes; x3 on gpsimd+scalar
            consts = load_consts()
            qws = [load_qw(i) for i in range(min(2, AHEAD, n_otiles))]
            load_x()
            prep = [prep_otile(i, consts, qws[i]) for i in range(len(qws))]
            # qw2 deferred past otile-0's transposes: keeps the ring clear for
            # qw0's first chunk, still ~25us ahead of otile-2's matmuls
            if AHEAD > 2 and n_otiles > 2:
                prep.append(prep_otile(2, consts, load_qw(2)))
            for ot in range(n_otiles):
                if ot + AHEAD < n_otiles:
                    prep.append(prep_otile(ot + AHEAD, consts, load_qw(ot + AHEAD)))
                mm_otile(ot, prep.pop(0), consts, last=(ot == n_otiles - 1))

            # consume the warmup chain at the very end so it can't be elided
            wm_t = cpool.tile([128, 1], F32, tag="wm")
            nc.vector.tensor_copy(wm_t[:], dps_t[:, :1])
            nc.gpsimd.dma_start(warm[:], wm_t[:])

    nc.compile()
    return nc


def _get_nc(m=M, k=IN, oc=OC, n_cores=N_CORES):
    key = (m, k, oc, n_cores)
    if key not in _CACHE:
        _CACHE[key] = _build(*key)
    return _CACHE[key]


def _make_in_maps(x, qweight, qzeros, scales, bias, n_cores=N_CORES):
    iw8 = _unpack_int4(qweight)  # [IN, OUT] uint8
    iz8 = _unpack_int4(qzeros)  # [G, OUT] uint8
    kk, mm = x.shape[1], x.shape[0]
    # [p, group, m]: partition-major so each partition's slab is contiguous
    x3 = np.ascontiguousarray(x.T.reshape(kk // GS, GS, mm).transpose(1, 0, 2))
    oc = qweight.shape[1] * PACK // n_cores
    n_ot = (oc + 127) // 128
    ocp = n_ot * 128

    def pm(a):
        # pad rows to whole otiles, then [ocp, d] -> [128, n_ot, d]
        a = np.pad(a, [(0, ocp - oc)] + [(0, 0)] * (a.ndim - 1))
        return np.ascontiguousarray(a.reshape(n_ot, 128, -1).transpose(1, 0, 2))

    in_maps = []
    for c in range(n_cores):
        sl = slice(c * oc, (c + 1) * oc)
        qw8T = np.ascontiguousarray(iw8[:, sl].T)
        in_maps.append(
            {
                "x3": x3,
                "qw8T": qw8T,
                "qw0c": np.ascontiguousarray(qw8T[:128, :2048]),
                "sT": pm(scales[:, sl].T.astype(np.float32)),
                "z8T": pm(iz8[:, sl].T),
                "biasT": pm(bias[sl].reshape(-1, 1).astype(np.float32)),
            }
        )
    return in_maps


LAST_EXEC_NS = None


def kernel(x, qweight, qzeros, scales, bias):
    global LAST_EXEC_NS
    from concourse.bass_utils import run_bass_kernel_spmd

    x = np.asarray(x)
    qweight = np.asarray(qweight)
    qzeros = np.asarray(qzeros)
    scales = np.asarray(scales)
    bias = np.asarray(bias)

    nc = _get_nc()
    in_maps = _make_in_maps(x, qweight, qzeros, scales, bias)

    kwargs = {}
    if os.environ.get("AWQ_PROFILE"):
        _enable_profiling()
        kwargs = dict(trace=True, tmpdir=os.environ.get("AWQ_TRACE_DIR") or None)
    res = run_bass_kernel_spmd(nc, in_maps, list(range(N_CORES)), **kwargs)
    LAST_EXEC_NS = res.exec_time_ns

    outT = np.concatenate([res.results[c]["outT"] for c in range(N_CORES)], axis=0)
    return np.ascontiguousarray(outT.T)


def _enable_profiling():
    """Register the NTFF profile hook missing from this image's antenv."""
    import types

    if "antenv.axon_hooks" not in sys.modules:
        import antenv

        mod = types.ModuleType("antenv.axon_hooks")
        mod._hook = None
        mod.set_axon_ntff_profile_hook = lambda h: setattr(mod, "_hook", h)
        mod.get_axon_ntff_profile_hook = lambda: mod._hook
        sys.modules["antenv.axon_hooks"] = mod
        antenv.axon_hooks = mod
        try:
            from trn_agent_boot.trn_boot import _ntff_profile_via_ctypes

            mod.set_axon_ntff_profile_hook(
                _ntff_profile_via_ctypes("/opt/axon/libaxon_pjrt.so")
            )
        except Exception:
            pass
    import concourse.bass_utils as _bu

    _bu.upload_artifacts = lambda tmpdir: "local://skipped"


# revision 4
# speedup vs baseline: 1.0482x; 1.0482x over previous
"""AWQ 4-bit quantized linear (group size 128) on 8 Trainium2 NeuronCores.

Column-parallel: each core owns OUT/8 = 1376 output columns. The host does
layout-only prep (slicing, int4->uint8 nibble widening with the AWQ column
permutation, transposes); all arithmetic — zero-point subtract, scale
multiply, matmul, bias — runs on device.

Per-core device pipeline, for each 128-row block of output columns (o-tile):
  1. DMA the packed-weight rows (uint8 nibbles, o on partitions).
  2. DVE tensor_scalar dual-op dequant: w = nib * s[o] - (z*s)[o]  (fp16 out).
     Scales/zeros vary per (o, group); with o on partitions they are
     per-partition scalars, which tensor_scalar supports natively.
  3. DMA xbar transpose each [o=128, k=128] fp16 tile to [k=128, o=128]
     (matmul needs the contraction dim on partitions).
  4. PE matmul accumulation over the 32 k-groups into PSUM:
     outT[o, m] += w_g[k, o].T @ xT_g[k, m].
  5. ACT/DVE evacuation: out = psum + bias[o] -> fp16, DMA to DRAM.

Head optimizations (the PE stream floor is 704 MMs x 216 ns = 152 us; all
wins beyond that are in not stalling it):
  - ~52 dummy matmuls on zeroed SBUF issued first: lifts the HAM clock gate
    to K=8/8 (~3.4 us of sustained PE activity) before real MMs arrive, and
    keeps the PE warm through the DMA-bound head.
  - consts go on the gpsimd ring so the sync ring starts the otile-0 weight
    chunk immediately; otile-0's first 2 KB/partition of packed weights come
    from a host-contiguous copy (qw0c) for faster DMA.
  - otile 0 dequants in 8 chunks of 4 groups (others: 4 chunks of 8) so the
    first transpose and first matmul land as early as possible.
  - x3 slab loads alternate between the gpsimd and scalar DMA rings.
"""

import os
import sys

import numpy as np

if "/opt/trn_rl_repo" not in sys.path:
    sys.path.insert(0, "/opt/trn_rl_repo")

M, IN, OUT = 1024, 4096, 11008
N_CORES = 8
OC = OUT // N_CORES  # 1376 output columns per core
GS = 128  # quantization group size (== matmul k-tile)
G = IN // GS  # 32 groups
PACK = 8  # int4 values per int32 word
# reference unpacks nibble k to logical column AWQ_REVERSE_ORDER.index(k);
# equivalently logical column j within a word uses shift 4*REV[j]:
REV = np.array([0, 4, 1, 5, 2, 6, 3, 7], dtype=np.uint32)

MM_N = 512  # moving-operand free size per matmul (one PSUM bank of fp32)
N_DUMMY = 40  # PE warmup matmuls (~3.4us cold + ~7us warm coverage)

_CACHE = {}


def _unpack_int4(q: np.ndarray) -> np.ndarray:
    """[rows, cols//8] int32 -> [rows, cols] uint8 in 0..15 (AWQ order)."""
    qu = q.view(np.uint32)
    nib = (qu[:, :, None] >> (REV * 4)[None, None, :]) & 0xF
    return nib.reshape(q.shape[0], -1).astype(np.uint8)


def _build(m, k, oc, n_cores):
    import concourse.bacc as bacc
    import concourse.tile as tile
    from concourse import mybir

    F16 = mybir.dt.float16
    F32 = mybir.dt.float32
    U8 = mybir.dt.uint8
    IDENT = mybir.ActivationFunctionType.Identity

    g = k // GS
    n_otiles = (oc + 127) // 128
    n_mch = (m + MM_N - 1) // MM_N

    nc = bacc.Bacc("TRN2", target_bir_lowering=False, debug=False)
    # x pre-swizzled on host to the SBUF layout [partition, group, m] so the
    # load runs with contiguous multi-KB per-partition descriptors
    x3 = nc.dram_tensor("x3", [128, k // GS, m], F16, kind="ExternalInput").ap()
    qw8T = nc.dram_tensor("qw8T", [oc, k], U8, kind="ExternalInput").ap()
    # first 2KB/partition of otile 0 as a contiguous block for the fastest
    # possible first-chunk DMA (dequant critical path at the head)
    qw0c = nc.dram_tensor("qw0c", [128, 2048], U8, kind="ExternalInput").ap()
    n_ot = n_otiles
    # per-otile constants pre-swizzled host-side to partition-major
    # [128, otile, group] so each loads as one contiguous run per partition
    sT = nc.dram_tensor("sT", [128, n_ot, g], F32, kind="ExternalInput").ap()
    z8T = nc.dram_tensor("z8T", [128, n_ot, g], U8, kind="ExternalInput").ap()
    biasT = nc.dram_tensor("biasT", [128, n_ot, 1], F32, kind="ExternalInput").ap()
    outT = nc.dram_tensor("outT", [oc, m], F16, kind="ExternalOutput").ap()
    warm = nc.dram_tensor("warm", [128, 1], F32, kind="ExternalOutput").ap()

    # dequant split 5 DVE / 3 ACT per 8 groups
    # (GPSIMD intentionally unused: it port-muxes with DVE and is ~7x slower)
    deq_pattern = [0, 1, 0, 0, 1, 0, 0, 1]

    with tile.TileContext(nc) as tc:
        with (
            tc.tile_pool(name="x", bufs=1) as xpool,
            tc.tile_pool(name="consts", bufs=1) as cpool,
            tc.tile_pool(name="qw", bufs=4) as qwpool,
            tc.tile_pool(name="wd", bufs=16) as wdpool,
            tc.tile_pool(name="w", bufs=16) as wpool,
            tc.tile_pool(name="ps", bufs=7, space="PSUM") as pspool,
            tc.tile_pool(name="wps", bufs=1, space="PSUM") as wmpool,
            tc.tile_pool(name="o", bufs=4) as opool,
        ):
            # resident transposed activations: [128, g, m]
            xT_sb = xpool.tile([128, g, m], F16)

            # PE warmup: dummy matmuls on a zeroed tile, no data deps. The
            # result is read once at the very end (warm output) so the chain
            # cannot be considered dead.
            dmy_t = cpool.tile([128, MM_N], F16, tag="dmy")
            dps_t = wmpool.tile([128, MM_N], F32, tag="wps")
            nc.gpsimd.memset(dmy_t[:], 0.0)
            for _ in range(N_DUMMY):
                nc.tensor.matmul(
                    dps_t[:], dmy_t[:, :128], dmy_t[:],
                    start=True, stop=True, skip_group_check=True,
                )

            def load_consts():
                s_t = cpool.tile([128, n_otiles, g], F32, tag="sT")
                z8_t = cpool.tile([128, n_otiles, g], U8, tag="z8")
                zf_t = cpool.tile([128, n_otiles, g], F32, tag="zf")
                zs_t = cpool.tile([128, n_otiles, g], F32, tag="zs")
                nzs_t = cpool.tile([128, n_otiles, g], F32, tag="nzs")
                b_t = cpool.tile([128, n_otiles, 1], F32, tag="b")
                nc.sync.dma_start(s_t[:], sT[:])
                nc.sync.dma_start(z8_t[:], z8T[:])
                nc.sync.dma_start(b_t[:], biasT[:])
                nc.vector.tensor_copy(zf_t[:], z8_t[:])
                nc.vector.tensor_tensor(
                    zs_t[:], zf_t[:], s_t[:], mybir.AluOpType.mult
                )
                # on ACT: also warms the activation table before dequant needs it
                nc.scalar.activation(nzs_t[:], zs_t[:], IDENT, scale=-1.0)
                return s_t, zs_t, nzs_t, b_t

            def load_x_head():
                # first half of x on the scalar HWDGE ring (fast; the gpsimd
                # SWDGE ring generates descriptors in microcode and is slow)
                for g0 in (0, 4, 8, 12):
                    nc.scalar.dma_start(xT_sb[:, g0 : g0 + 4], x3[:, g0 : g0 + 4])

            def load_x_tail():
                # second half on sync, behind the otile-0/1 weight loads
                for g0 in (16, 24):
                    nc.sync.dma_start(xT_sb[:, g0 : g0 + 8], x3[:, g0 : g0 + 8])

            def chunk_groups(ot):
                # otile 0 uses finer chunks so its first matmuls unblock early
                return (4, 8) if ot == 0 else (8, 4)

            def load_qw(ot):
                o0 = ot * 128
                ob = min(128, oc - o0)
                qw_t = qwpool.tile([ob, k], U8, name=f"qw_{ot}", tag="qw")
                if ot == 0:
                    nc.sync.dma_start(qw_t[:, 0:512], qw0c[:, 0:512])
                    nc.sync.dma_start(qw_t[:, 512:2048], qw0c[:, 512:2048])
                    nc.sync.dma_start(qw_t[:, 2048:k], qw8T[o0 : o0 + ob, 2048:k])
                else:
                    nc.sync.dma_start(qw_t[:], qw8T[o0 : o0 + ob])
                return qw_t

            def prep_otile(ot, consts, qw_t):
                o0 = ot * 128
                ob = min(128, oc - o0)
                s_all, zs_all, nzs_all, _ = consts
                s_t = s_all[:ob, ot]
                zs_t = zs_all[:ob, ot]
                nzs_t = nzs_all[:ob, ot]
                qg, nch = chunk_groups(ot)

                # dequant + transpose per chunk (qg groups each)
                w3_qs = []
                for q in range(nch):
                    wd_t = wdpool.tile([ob, qg * GS], F16, tag="wd")
                    for j in range(qg):
                        gi = q * qg + j
                        ksl_q = slice(j * GS, (j + 1) * GS)
                        ksl = slice(gi * GS, (gi + 1) * GS)
                        if deq_pattern[gi % len(deq_pattern)]:
                            nc.scalar.activation(
                                wd_t[:, ksl_q],
                                qw_t[:, ksl],
                                IDENT,
                                bias=nzs_t[:, gi : gi + 1],
                                scale=s_t[:, gi : gi + 1],
                            )
                        else:
                            nc.vector.tensor_scalar(
                                wd_t[:, ksl_q],
                                qw_t[:, ksl],
                                s_t[:, gi : gi + 1],
                                zs_t[:, gi : gi + 1],
                                mybir.AluOpType.mult,
                                mybir.AluOpType.subtract,
                            )
                    # xbar transpose [ob, qg*GS] -> [128, qg, ob]
                    w3_t = wpool.tile([128, qg, ob], F16, tag="w")
                    nc.sync.dma_start_transpose(w3_t[:], wd_t[:])
                    w3_qs.append(w3_t)
                return o0, ob, qg, w3_qs

            def mm_otile(ot, prep, consts, last):
                # interleave the m-chunk accumulation chains per group so each
                # weight chunk is consumed in one dense burst of PE work
                o0, ob, qg, w3_qs = prep
                b_t = consts[3][:ob, ot]
                mslices = [
                    slice(mc * MM_N, min(m, (mc + 1) * MM_N)) for mc in range(n_mch)
                ]
                pss = [
                    pspool.tile([ob, MM_N], F32, name=f"ps_{ot}_{mc}", tag="ps")
                    for mc in range(n_mch)
                ]
                for gi in range(g):
                    for mc in range(n_mch):
                        msl = mslices[mc]
                        nc.tensor.matmul(
                            pss[mc][:, : msl.stop - msl.start],
                            w3_qs[gi // qg][:, gi % qg, :],
                            xT_sb[:, gi, msl],
                            start=(gi == 0),
                            stop=(gi == g - 1),
                        )
                for mc in range(n_mch):
                    msl = mslices[mc]
                    mn = msl.stop - msl.start
                    o_t = opool.tile([ob, MM_N], F16)
                    if not last:
                        # evac split across ACT and DVE to balance engine load
                        if mc == 0:
                            nc.scalar.activation(
                                o_t[:, :mn], pss[mc][:, :mn], IDENT,
                                bias=b_t[:], scale=1.0,
                            )
                        else:
                            nc.vector.tensor_scalar_add(
                                o_t[:, :mn], pss[mc][:, :mn], b_t[:]
                            )
                        nc.gpsimd.dma_start(outT[o0 : o0 + ob, msl], o_t[:, :mn])
                    else:
                        # tail: pipeline evac+store in half-chunks on two rings
                        h = mn // 2
                        for hf in range(2):
                            osl = slice(hf * h, (hf + 1) * h)
                            hsl = slice(msl.start + hf * h, msl.start + (hf + 1) * h)
                            if hf == 0:
                                nc.scalar.activation(
                                    o_t[:, osl], pss[mc][:, osl], IDENT,
                                    bias=b_t[:], scale=1.0,
                                )
                                nc.sync.dma_start(outT[o0 : o0 + ob, hsl], o_t[:, osl])
                            else:
                                nc.vector.tensor_scalar_add(
                                    o_t[:, osl], pss[mc][:, osl], b_t[:]
                                )
                                nc.sync.dma_start(
                                    outT[o0 : o0 + ob, hsl], o_t[:, osl]
                                )

            AHEAD = 3
            # issue order: consts on gpsimd; otile-0/1 packed weights on sync
            # ahead of the (data-dependent) transposes; x3 on gpsimd+scalar
            consts = load_consts()
            qws = [load_qw(i) for i in range(min(2, AHEAD, n_otiles))]
            load_x_head()
            load_x_tail()
            prep = [prep_otile(i, consts, qws[i]) for i in range(len(qws))]
            # qw2 deferred past otile-0's transposes: keeps the ring clear for
            # qw0's first chunk, still ~25us ahead of otile-2's matmuls
            if AHEAD > 2 and n_otiles > 2:
                prep.append(prep_otile(2, consts, load_qw(2)))
            for ot in range(n_otiles):
                if ot + AHEAD < n_otiles:
                    prep.append(prep_otile(ot + AHEAD, consts, load_qw(ot + AHEAD)))
                mm_otile(ot, prep.pop(0), consts, last=(ot == n_otiles - 1))

            # consume the warmup chain at the very end so it can't be elided
            wm_t = cpool.tile([128, 1], F32, tag="wm")
            nc.vector.tensor_copy(wm_t[:], dps_t[:, :1])
            nc.gpsimd.dma_start(warm[:], wm_t[:])

    nc.compile()
    return nc


def _get_nc(m=M, k=IN, oc=OC, n_cores=N_CORES):
    key = (m, k, oc, n_cores)
    if key not in _CACHE:
        _CACHE[key] = _build(*key)
    return _CACHE[key]


def _make_in_maps(x, qweight, qzeros, scales, bias, n_cores=N_CORES):
    iw8 = _unpack_int4(qweight)  # [IN, OUT] uint8
    iz8 = _unpack_int4(qzeros)  # [G, OUT] uint8
    kk, mm = x.shape[1], x.shape[0]
    # [p, group, m]: partition-major so each partition's slab is contiguous
    x3 = np.ascontiguousarray(x.T.reshape(kk // GS, GS, mm).transpose(1, 0, 2))
    oc = qweight.shape[1] * PACK // n_cores
    n_ot = (oc + 127) // 128
    ocp = n_ot * 128

    def pm(a):
        # pad rows to whole otiles, then [ocp, d] -> [128, n_ot, d]
        a = np.pad(a, [(0, ocp - oc)] + [(0, 0)] * (a.ndim - 1))
        return np.ascontiguousarray(a.reshape(n_ot, 128, -1).transpose(1, 0, 2))

    in_maps = []
    for c in range(n_cores):
        sl = slice(c * oc, (c + 1) * oc)
        qw8T = np.ascontiguousarray(iw8[:, sl].T)
        in_maps.append(
            {
                "x3": x3,
                "qw8T": qw8T,
                "qw0c": np.ascontiguousarray(qw8T[:128, :2048]),
                "sT": pm(scales[:, sl].T.astype(np.float32)),
                "z8T": pm(iz8[:, sl].T),
                "biasT": pm(bias[sl].reshape(-1, 1).astype(np.float32)),
            }
        )
    return in_maps


LAST_EXEC_NS = None


def kernel(x, qweight, qzeros, scales, bias):
    global LAST_EXEC_NS
    from concourse.bass_utils import run_bass_kernel_spmd

    x = np.asarray(x)
    qweight = np.asarray(qweight)
    qzeros = np.asarray(qzeros)
    scales = np.asarray(scales)
    bias = np.asarray(bias)

    nc = _get_nc()
    in_maps = _make_in_maps(x, qweight, qzeros, scales, bias)

    kwargs = {}
    if os.environ.get("AWQ_PROFILE"):
        _enable_profiling()
        kwargs = dict(trace=True, tmpdir=os.environ.get("AWQ_TRACE_DIR") or None)
    res = run_bass_kernel_spmd(nc, in_maps, list(range(N_CORES)), **kwargs)
    LAST_EXEC_NS = res.exec_time_ns

    outT = np.concatenate([res.results[c]["outT"] for c in range(N_CORES)], axis=0)
    return np.ascontiguousarray(outT.T)


def _enable_profiling():
    """Register the NTFF profile hook missing from this image's antenv."""
    import types

    if "antenv.axon_hooks" not in sys.modules:
        import antenv

        mod = types.ModuleType("antenv.axon_hooks")
        mod._hook = None
        mod.set_axon_ntff_profile_hook = lambda h: setattr(mod, "_hook", h)
        mod.get_axon_ntff_profile_hook = lambda: mod._hook
        sys.modules["antenv.axon_hooks"] = mod
        antenv.axon_hooks = mod
        try:
            from trn_agent_boot.trn_boot import _ntff_profile_via_ctypes

            mod.set_axon_ntff_profile_hook(
                _ntff_profile_via_ctypes("/opt/axon/libaxon_pjrt.so")
            )
        except Exception:
            pass
    import concourse.bass_utils as _bu

    _bu.upload_artifacts = lambda tmpdir: "local://skipped"


# revision 5
# speedup vs baseline: 1.0955x; 1.0451x over previous
"""AWQ 4-bit quantized linear (group size 128) on 8 Trainium2 NeuronCores.

Column-parallel: each core owns OUT/8 = 1376 output columns. The host does
layout-only prep (slicing, int4->uint8 nibble widening with the AWQ column
permutation, padding, reshapes); all arithmetic — zero-point subtract, scale
multiply, matmul, bias — runs on device.

K-major, transpose-free design. The PE stream floor is 704 matmuls x 216 ns
= 152 us; everything else is arranged to never stall it:

  - Packed weights arrive k-major ([kp=128, group, o] per otile), which IS
    the matmul stationary layout — no on-device transposes. (The previous
    o-major design needed 44 xbar-transpose DMAs whose ~358-byte packet
    storms monopolized the 16 shared DMA engines and starved the x load.)
  - Scales/zeros vary along the free (o) axis in this layout, so they are
    partition-broadcast via stride-0-source DMAs (8 KB read -> 1 MB write),
    and dequant is two full-otile DVE tensor_tensor passes:
      qz = q - z_bc (exact small ints in fp16), w = qz * s_bc.
  - ~28 dummy matmuls on a zeroed tile run first: lifts the HAM clock gate
    to K=8/8 (~3.4 us of sustained PE activity) and keeps the PE warm
    through the DMA-bound head.
  - Ring assignment: sync (HWDGE) carries per-otile weights + broadcasts +
    the tail stores; scalar (HWDGE) carries the resident x slabs; the slow
    gpsimd SWDGE ring only carries latency-tolerant mid-kernel out stores.
  - PE matmul accumulation over the 32 k-groups into PSUM:
      outT[o, m] += w[kp, g, o].T @ xT[kp, g, m]
    evacuated with bias via ACT (m-chunk 0) and DVE (m-chunk 1).
"""

import os
import sys

import numpy as np

if "/opt/trn_rl_repo" not in sys.path:
    sys.path.insert(0, "/opt/trn_rl_repo")

M, IN, OUT = 1024, 4096, 11008
N_CORES = 8
OC = OUT // N_CORES  # 1376 output columns per core
GS = 128  # quantization group size (== matmul k-tile)
G = IN // GS  # 32 groups
PACK = 8  # int4 values per int32 word
# reference unpacks nibble k to logical column AWQ_REVERSE_ORDER.index(k);
# equivalently logical column j within a word uses shift 4*REV[j]:
REV = np.array([0, 4, 1, 5, 2, 6, 3, 7], dtype=np.uint32)

MM_N = 512  # moving-operand free size per matmul (one PSUM bank of fp32)
N_DUMMY = 28  # PE warmup matmuls (~3.4us cold + ~4.3us warm coverage)
QC = 8  # groups per dequant chunk (pipelining granularity)

_CACHE = {}


def _unpack_int4(q: np.ndarray) -> np.ndarray:
    """[rows, cols//8] int32 -> [rows, cols] uint8 in 0..15 (AWQ order)."""
    qu = q.view(np.uint32)
    nib = (qu[:, :, None] >> (REV * 4)[None, None, :]) & 0xF
    return nib.reshape(q.shape[0], -1).astype(np.uint8)


def _build(m, k, oc, n_cores):
    import concourse.bacc as bacc
    import concourse.tile as tile
    from concourse import mybir

    F16 = mybir.dt.float16
    F32 = mybir.dt.float32
    U8 = mybir.dt.uint8
    IDENT = mybir.ActivationFunctionType.Identity

    g = k // GS
    n_otiles = (oc + 127) // 128
    n_mch = (m + MM_N - 1) // MM_N
    n_qc = g // QC

    nc = bacc.Bacc("TRN2", target_bir_lowering=False, debug=False)
    # x pre-swizzled on host to the SBUF layout [partition, group, m] so the
    # load runs with contiguous multi-KB per-partition descriptors
    x3 = nc.dram_tensor("x3", [128, g, m], F16, kind="ExternalInput").ap()
    # k-major packed weights: [otile, kp, group, o] — per-partition runs of
    # 4 KB per otile, and the dequant output needs no transpose
    qwK = nc.dram_tensor("qwK", [n_otiles, 128, g, 128], U8, kind="ExternalInput").ap()
    # per-otile scale / zero rows, contiguous so the broadcast DMA reads one
    # 8 KB (resp 4 KB) run per partition
    sKot = nc.dram_tensor("sKot", [n_otiles, 1, g, 128], F16, kind="ExternalInput").ap()
    zKot = nc.dram_tensor("zKot", [n_otiles, 1, g, 128], U8, kind="ExternalInput").ap()
    biasT = nc.dram_tensor("biasT", [128, n_otiles, 1], F32, kind="ExternalInput").ap()
    outT = nc.dram_tensor("outT", [oc, m], F16, kind="ExternalOutput").ap()
    warm = nc.dram_tensor("warm", [128, 1], F32, kind="ExternalOutput").ap()

    with tile.TileContext(nc) as tc:
        with (
            tc.tile_pool(name="x", bufs=1) as xpool,
            tc.tile_pool(name="consts", bufs=1) as cpool,
            tc.tile_pool(name="q", bufs=3) as qpool,
            tc.tile_pool(name="sbc", bufs=2) as sbcpool,
            tc.tile_pool(name="zbc", bufs=2) as zbcpool,
            tc.tile_pool(name="qz", bufs=6) as qzpool,
            tc.tile_pool(name="w", bufs=4) as wpool,
            tc.tile_pool(name="ps", bufs=7, space="PSUM") as pspool,
            tc.tile_pool(name="wps", bufs=1, space="PSUM") as wmpool,
            tc.tile_pool(name="o", bufs=4) as opool,
        ):
            # resident transposed activations: [128, g, m]
            xT_sb = xpool.tile([128, g, m], F16)

            # PE warmup: dummy matmuls on a zeroed tile, no data deps. The
            # result is read once at the very end (warm output) so the chain
            # cannot be considered dead.
            dmy_t = cpool.tile([128, MM_N], F16, tag="dmy")
            dps_t = wmpool.tile([128, MM_N], F32, tag="wps")
            nc.gpsimd.memset(dmy_t[:], 0.0)
            for _ in range(N_DUMMY):
                nc.tensor.matmul(
                    dps_t[:], dmy_t[:, :128], dmy_t[:],
                    start=True, stop=True, skip_group_check=True,
                )

            def load_consts():
                b_t = cpool.tile([128, n_otiles, 1], F32, tag="b")
                nc.sync.dma_start(b_t[:], biasT[:])
                return b_t

            def load_x(slabs):
                for g0, g1 in slabs:
                    nc.scalar.dma_start(xT_sb[:, g0:g1], x3[:, g0:g1])

            def prep_otile(ot):
                # weights + broadcast consts on sync; two-pass dequant on DVE
                q_t = qpool.tile([128, g, 128], U8, tag="q")
                if ot == 0:
                    # split so the first chunk's dequant starts earlier
                    nc.sync.dma_start(q_t[:, 0:QC], qwK[ot, :, 0:QC])
                    nc.sync.dma_start(q_t[:, QC:g], qwK[ot, :, QC:g])
                else:
                    nc.sync.dma_start(q_t[:], qwK[ot])
                s_bc = sbcpool.tile([128, 1, g, 128], F16, tag="sbc")
                z_bc = zbcpool.tile([128, 1, g, 128], U8, tag="zbc")
                nc.sync.dma_start(s_bc[:], sKot[ot].partition_broadcast(128))
                nc.sync.dma_start(z_bc[:], zKot[ot].partition_broadcast(128))
                w_t = wpool.tile([128, g, 128], F16, tag="w")
                for c in range(n_qc):
                    gsl = slice(c * QC, (c + 1) * QC)
                    qz_t = qzpool.tile([128, QC, 128], F16, tag="qz")
                    nc.vector.tensor_tensor(
                        qz_t[:], q_t[:, gsl], z_bc[:, 0, gsl],
                        mybir.AluOpType.subtract,
                    )
                    nc.vector.tensor_tensor(
                        w_t[:, gsl], qz_t[:], s_bc[:, 0, gsl],
                        mybir.AluOpType.mult,
                    )
                return w_t

            def mm_otile(ot, w_t, b_all, last):
                # interleave the m-chunk accumulation chains per group so each
                # dequant chunk is consumed in one dense burst of PE work
                o0 = ot * 128
                ob = min(128, oc - o0)
                b_t = b_all[:ob, ot]
                mslices = [
                    slice(mc * MM_N, min(m, (mc + 1) * MM_N)) for mc in range(n_mch)
                ]
                pss = [
                    pspool.tile([128, MM_N], F32, name=f"ps_{ot}_{mc}", tag="ps")
                    for mc in range(n_mch)
                ]
                for gi in range(g):
                    for mc in range(n_mch):
                        msl = mslices[mc]
                        nc.tensor.matmul(
                            pss[mc][:, : msl.stop - msl.start],
                            w_t[:, gi, :],
                            xT_sb[:, gi, msl],
                            start=(gi == 0),
                            stop=(gi == g - 1),
                        )
                for mc in range(n_mch):
                    msl = mslices[mc]
                    mn = msl.stop - msl.start
                    o_t = opool.tile([128, MM_N], F16)
                    if not last:
                        # evac split across ACT and DVE to balance engine load
                        if mc == 0:
                            nc.scalar.activation(
                                o_t[:ob, :mn], pss[mc][:ob, :mn], IDENT,
                                bias=b_t[:], scale=1.0,
                            )
                        else:
                            nc.vector.tensor_scalar_add(
                                o_t[:ob, :mn], pss[mc][:ob, :mn], b_t[:]
                            )
                        nc.gpsimd.dma_start(outT[o0 : o0 + ob, msl], o_t[:ob, :mn])
                    else:
                        # tail: pipeline evac+store in half-chunks on the
                        # fast sync ring
                        h = mn // 2
                        for hf in range(2):
                            osl = slice(hf * h, (hf + 1) * h)
                            hsl = slice(msl.start + hf * h, msl.start + (hf + 1) * h)
                            if hf == 0:
                                nc.scalar.activation(
                                    o_t[:ob, osl], pss[mc][:ob, osl], IDENT,
                                    bias=b_t[:], scale=1.0,
                                )
                            else:
                                nc.vector.tensor_scalar_add(
                                    o_t[:ob, osl], pss[mc][:ob, osl], b_t[:]
                                )
                            nc.sync.dma_start(outT[o0 : o0 + ob, hsl], o_t[:ob, osl])

            AHEAD = 3
            b_all = load_consts()
            load_x([(0, 4), (4, 8)])
            prep = [prep_otile(0)]
            load_x([(8, 16), (16, 24), (24, 32)])
            for i in range(1, min(AHEAD, n_otiles)):
                prep.append(prep_otile(i))
            for ot in range(n_otiles):
                if ot + AHEAD < n_otiles:
                    prep.append(prep_otile(ot + AHEAD))
                mm_otile(ot, prep.pop(0), b_all, last=(ot == n_otiles - 1))

            # consume the warmup chain at the very end so it can't be elided
            wm_t = cpool.tile([128, 1], F32, tag="wm")
            nc.vector.tensor_copy(wm_t[:], dps_t[:, :1])
            nc.gpsimd.dma_start(warm[:], wm_t[:])

    nc.compile()
    return nc


def _get_nc(m=M, k=IN, oc=OC, n_cores=N_CORES):
    key = (m, k, oc, n_cores)
    if key not in _CACHE:
        _CACHE[key] = _build(*key)
    return _CACHE[key]


def _make_in_maps(x, qweight, qzeros, scales, bias, n_cores=N_CORES):
    iw8 = _unpack_int4(qweight)  # [IN, OUT] uint8
    iz8 = _unpack_int4(qzeros)  # [G, OUT] uint8
    kk, mm = x.shape[1], x.shape[0]
    # [p, group, m]: partition-major so each partition's slab is contiguous
    x3 = np.ascontiguousarray(x.T.reshape(kk // GS, GS, mm).transpose(1, 0, 2))
    oc = qweight.shape[1] * PACK // n_cores
    n_ot = (oc + 127) // 128
    ocp = n_ot * 128
    g = kk // GS

    def padc(a):
        # pad the o (last) axis to whole otiles
        return np.pad(a, [(0, 0)] * (a.ndim - 1) + [(0, ocp - oc)])

    def pm(a):
        # pad rows to whole otiles, then [ocp, d] -> [128, n_ot, d]
        a = np.pad(a, [(0, ocp - oc)] + [(0, 0)] * (a.ndim - 1))
        return np.ascontiguousarray(a.reshape(n_ot, 128, -1).transpose(1, 0, 2))

    in_maps = []
    for c in range(n_cores):
        sl = slice(c * oc, (c + 1) * oc)
        # [k, o] -> [g, kp, n_ot, o128] -> [n_ot, kp, g, o128]
        iw = padc(iw8[:, sl]).reshape(g, GS, n_ot, 128)
        qwK = np.ascontiguousarray(iw.transpose(2, 1, 0, 3))
        s = padc(scales[:, sl]).reshape(g, n_ot, 128)
        sKot = np.ascontiguousarray(s.transpose(1, 0, 2))[:, None]
        z = padc(iz8[:, sl]).reshape(g, n_ot, 128)
        zKot = np.ascontiguousarray(z.transpose(1, 0, 2))[:, None]
        in_maps.append(
            {
                "x3": x3,
                "qwK": qwK,
                "sKot": sKot,
                "zKot": zKot,
                "biasT": pm(bias[sl].reshape(-1, 1).astype(np.float32)),
            }
        )
    return in_maps


LAST_EXEC_NS = None


def kernel(x, qweight, qzeros, scales, bias):
    global LAST_EXEC_NS
    from concourse.bass_utils import run_bass_kernel_spmd

    x = np.asarray(x)
    qweight = np.asarray(qweight)
    qzeros = np.asarray(qzeros)
    scales = np.asarray(scales)
    bias = np.asarray(bias)

    nc = _get_nc()
    in_maps = _make_in_maps(x, qweight, qzeros, scales, bias)

    kwargs = {}
    if os.environ.get("AWQ_PROFILE"):
        _enable_profiling()
        kwargs = dict(trace=True, tmpdir=os.environ.get("AWQ_TRACE_DIR") or None)
    res = run_bass_kernel_spmd(nc, in_maps, list(range(N_CORES)), **kwargs)
    LAST_EXEC_NS = res.exec_time_ns

    outT = np.concatenate([res.results[c]["outT"] for c in range(N_CORES)], axis=0)
    return np.ascontiguousarray(outT.T)


def _enable_profiling():
    """Register the NTFF profile hook missing from this image's antenv."""
    import types

    if "antenv.axon_hooks" not in sys.modules:
        import antenv

        mod = types.ModuleType("antenv.axon_hooks")
        mod._hook = None
        mod.set_axon_ntff_profile_hook = lambda h: setattr(mod, "_hook", h)
        mod.get_axon_ntff_profile_hook = lambda: mod._hook
        sys.modules["antenv.axon_hooks"] = mod
        antenv.axon_hooks = mod
        try:
            from trn_agent_boot.trn_boot import _ntff_profile_via_ctypes

            mod.set_axon_ntff_profile_hook(
                _ntff_profile_via_ctypes("/opt/axon/libaxon_pjrt.so")
            )
        except Exception:
            pass
    import concourse.bass_utils as _bu

    _bu.upload_artifacts = lambda tmpdir: "local://skipped"
